# revision 13
# baseline (speedup 1.0000x reference)
"""GraphConvolution kernel for Trainium2 (8 NeuronCores, SPMD).

out = segment_sum(edge_w * (x @ W)[edge_src], edge_dst) + b

Strategy (graph/data parallel, dst-sharded):
  - Each core owns a contiguous shard of 12500 destination nodes, split
    into 98 dst blocks of 128. W commutes with segment_sum, so each core
    gathers raw x rows (bf16), accumulates pre[d, :] = sum_e w_e *
    x[src_e, :] per dst block via selection-matrix matmuls in PSUM (f32),
    then applies W per block and adds the bias.
  - The gather uses dma_gather (GPSIMD extended instruction, int16
    indices); x is addressed through 4 quartile tables of 25000 rows.
    Edges are bucketed per (dst block, quartile) cell. The kernel is
    Q7-descriptor-generation bound (~2.4 ns per gathered index once the
    four SWDGE queues are cycled: queue q runs its descgen on Q7 core
    pair (2q, 2q+1) and has its own descriptor ring, so consecutive
    calls on different queues never stall on one ring's drain).
  - x is cast to bf16 on the host (512B rows); selection matrices are
    built in bf16 on DVE with two broadcast tensor_tensor passes per
    (block, quartile) cell; PE accumulates in f32 PSUM.
  - Cell capacities are rounded to 16 indices; pad slots gather row 0
    and are nulled by zero columns (w=0) in the selection matrix. Each
    core orders its dst blocks by descending edge count (the output is
    un-permuted on the host) so the SPMD-uniform per-(rank, quartile)
    cap (max over cores) stays tight.
"""

import ml_dtypes
import numpy as np

import concourse.bass as bass
import concourse.bacc as bacc
import concourse.mybir as mybir
import concourse.tile as tile
from concourse.bass_utils import run_bass_kernel_spmd
from concourse.masks import make_identity

N_NODES = 100000
D_IN = 256
D_OUT = 128
N_CORES = 8
SHARD = N_NODES // N_CORES          # 12500 dst rows per core
P = 128
NBLK = (SHARD + P - 1) // P         # 98 dst blocks per core
OUT_ROWS = NBLK * P                 # 12544 padded output rows per core
NQ = 4
QROWS = (N_NODES + NQ - 1) // NQ    # 25000 rows per quartile table
GBUFS = 5                           # gather-tile buffering

last_exec_time_ns = None
last_results = None
_program_cache = {}


def _plan(caps16):
    """Derive static layout tables from the per-(rank, q) index counts."""
    chunks = (caps16 + P - 1) // P                 # [NBLK, NQ]
    qbase = np.zeros((NBLK, NQ), np.int64)
    qbase[:, 1:] = np.cumsum(chunks, axis=1)[:, :-1]
    c_rank = chunks.sum(axis=1)                    # chunks per ranked block
    rankbase = np.zeros(NBLK, np.int64)
    rankbase[1:] = np.cumsum(c_rank)[:-1]
    icols = caps16 // 16                           # idx columns per call
    ibase = np.zeros(NBLK * NQ + 1, np.int64)
    ibase[1:] = np.cumsum(icols.reshape(-1))
    return chunks, qbase, c_rank, rankbase, icols, ibase


def _build_program(caps_key):
    f32 = mybir.dt.float32
    bf16 = mybir.dt.bfloat16
    i16 = mybir.dt.int16
    caps16 = np.asarray(caps_key, np.int64).reshape(NBLK, NQ)
    chunks, qbase, c_rank, rankbase, icols, ibase = _plan(caps16)
    tot_chunks = int(c_rank.sum())
    tot_icols = int(ibase[-1])
    max_c = int(c_rank.max())
    max_cell = int(chunks.max())

    nc = bacc.Bacc("TRN2", target_bir_lowering=False, debug=False,
                   num_devices=N_CORES, num_swdge_queues=4)
    x_tbl = nc.dram_tensor("x_tbl", [N_NODES, D_IN], bf16,
                           kind="ExternalInput").ap()
    wmat = nc.dram_tensor("wmat", [D_IN, D_OUT], f32, kind="ExternalInput").ap()
    bbc = nc.dram_tensor("bbc", [P, D_OUT], f32, kind="ExternalInput").ap()
    idx = nc.dram_tensor("idx", [P, tot_icols], i16, kind="ExternalInput").ap()
    # mdst/mw are column-duplicated (pairs) so the broadcast APs in the mt
    # build keep an inner step-1 pair dim — required for DVE 2X_1PORT mode
    mdst = nc.dram_tensor("mdst", [P, 2 * tot_chunks], bf16,
                          kind="ExternalInput").ap()
    mw = nc.dram_tensor("mw", [P, 2 * tot_chunks], bf16,
                        kind="ExternalInput").ap()
    out = nc.dram_tensor("out", [OUT_ROWS, D_OUT], f32,
                         kind="ExternalOutput").ap()

    with tile.TileContext(nc) as tc:
        with (
            tc.tile_pool(name="const", bufs=1) as constp,
            tc.tile_pool(name="meta", bufs=1) as metap,
            tc.tile_pool(name="g", bufs=GBUFS) as gp,
            tc.tile_pool(name="eq", bufs=2) as eqp,
            tc.tile_pool(name="m", bufs=2) as mp,
            tc.tile_pool(name="pre", bufs=2, space="PSUM") as prep,
            tc.tile_pool(name="tp", bufs=2, space="PSUM") as tpp,
            tc.tile_pool(name="po", bufs=2, space="PSUM") as pop,
            tc.tile_pool(name="sb", bufs=3) as sbp,
            tc.tile_pool(name="st", bufs=4) as stp,
            tc.tile_pool(name="ob", bufs=3) as obp,
        ):
            w0 = constp.tile([P, D_OUT], f32, tag="w0")
            w1 = constp.tile([P, D_OUT], f32, tag="w1")
            nc.sync.dma_start(out=w0[:], in_=wmat[0:P, :])
            nc.sync.dma_start(out=w1[:], in_=wmat[P:2 * P, :])
            bb = constp.tile([P, D_OUT], f32, tag="bb")
            nc.sync.dma_start(out=bb[:], in_=bbc[:])
            iota_i = constp.tile([P, P], mybir.dt.int32, tag="ioi")
            nc.gpsimd.iota(iota_i[:], pattern=[[1, P]], base=0,
                           channel_multiplier=0)
            iota_b = constp.tile([P, P], bf16, tag="iob")
            nc.vector.tensor_copy(iota_b[:], iota_i[:])
            ident = constp.tile([P, P], f32, tag="id")
            make_identity(nc, ident[:])
            ones1 = constp.tile([1, P], f32, tag="on")
            nc.vector.memset(ones1[:], 1.0)

            idx_t = metap.tile([P, tot_icols], i16, tag="idx")
            mdst_t = metap.tile([P, 2 * tot_chunks], bf16, tag="mdst")
            mw_t = metap.tile([P, 2 * tot_chunks], bf16, tag="mw")
            # split the idx preload so the first gathers don't wait on the
            # whole table
            nsplit = 8
            csz = (tot_icols + nsplit - 1) // nsplit
            for s in range(nsplit):
                lo, hi = s * csz, min((s + 1) * csz, tot_icols)
                if lo < hi:
                    nc.sync.dma_start(out=idx_t[:, lo:hi], in_=idx[:, lo:hi])
            nc.sync.dma_start(out=mdst_t[:], in_=mdst[:])
            nc.sync.dma_start(out=mw_t[:], in_=mw[:])

            for r in range(NBLK):
                C = int(c_rank[r])
                gt = gp.tile([P, max_c * D_IN], bf16, tag="g")
                for q in range(NQ):
                    cap = int(caps16[r, q])
                    nch = int(chunks[r, q])
                    call = r * NQ + q
                    nc.gpsimd.dma_gather(
                        out_ap=gt[:, qbase[r, q] * D_IN:
                                  (qbase[r, q] + nch) * D_IN]
                        .rearrange("p (c d) -> p c d", d=D_IN),
                        in_ap=x_tbl[q * QROWS:(q + 1) * QROWS, :],
                        idxs_ap=idx_t[:, ibase[call]:ibase[call + 1]],
                        num_idxs=cap, num_idxs_reg=cap,
                        elem_size=D_IN, single_packet=False,
                        queue_num=q,
                    )
                pre = prep.tile([P, D_IN], f32, tag="pre")
                # selection matrices for the whole block, built in two
                # broadcast passes: eq[p, c, d] = (d == mdst[p, col0+c]),
                # mt = eq * mw. The pair-duplicated mdst/mw tables keep every
                # AP's innermost dim at step 1 (DVE 2X_1PORT eligibility).
                col0 = int(rankbase[r])
                eqt = eqp.tile([P, max_c * P], bf16, tag="eq")
                mt = mp.tile([P, max_c * P], bf16, tag="m")
                e4 = eqt[:, :C * P].rearrange("p (c e j) -> p c e j",
                                              e=P // 2, j=2)
                nc.vector.tensor_tensor(
                    out=e4,
                    in0=iota_b[:].rearrange("p (c e j) -> p c e j",
                                            c=1, j=2)
                    .to_broadcast([P, C, P // 2, 2]),
                    in1=mdst_t[:, 2 * col0:2 * (col0 + C)]
                    .rearrange("p (c e j) -> p c e j", e=1, j=2)
                    .to_broadcast([P, C, P // 2, 2]),
                    op=mybir.AluOpType.is_equal,
                )
                m4 = mt[:, :C * P].rearrange("p (c e j) -> p c e j",
                                             e=P // 2, j=2)
                nc.vector.tensor_tensor(
                    out=m4,
                    in0=e4,
                    in1=mw_t[:, 2 * col0:2 * (col0 + C)]
                    .rearrange("p (c e j) -> p c e j", e=1, j=2)
                    .to_broadcast([P, C, P // 2, 2]),
                    op=mybir.AluOpType.mult,
                )
                tot_c = 0
                for q in range(NQ):
                    cap = int(caps16[r, q])
                    nch = int(chunks[r, q])
                    for c in range(nch):
                        K = P if (c < nch - 1 or cap % P == 0) else cap % P
                        gcol = int(qbase[r, q]) + c
                        nc.tensor.matmul(
                            out=pre[:], lhsT=mt[:K, gcol * P:(gcol + 1) * P],
                            rhs=gt[:K, gcol * D_IN:(gcol + 1) * D_IN],
                            start=(tot_c == 0), stop=(tot_c == C - 1),
                        )
                        tot_c += 1
                assert tot_c == C
                sb_pre = sbp.tile([P, D_IN], f32, tag="sb")
                nc.scalar.copy(sb_pre[:], pre[:])
                po = pop.tile([P, D_OUT], f32, tag="po")
                nc.tensor.matmul(out=po[:], lhsT=ones1[:],
                                 rhs=bb[0:1, :], start=True, stop=False)
                for h in range(2):
                    pt = tpp.tile([P, P], f32, tag="pt")
                    nc.tensor.transpose(pt[:], sb_pre[:, h * P:(h + 1) * P],
                                        ident[:])
                    st = stp.tile([P, P], f32, tag="st")
                    nc.scalar.copy(st[:], pt[:])
                    nc.tensor.matmul(out=po[:], lhsT=st[:],
                                     rhs=(w0[:] if h == 0 else w1[:]),
                                     start=False, stop=(h == 1))
                ob = obp.tile([P, D_OUT], f32, tag="ob")
                nc.scalar.copy(ob[:], po[:])
                nc.sync.dma_start(out=out[r * P:(r + 1) * P, :], in_=ob[:])

    nc.compile()
    return nc


def _prep_inputs(x, edge_src, edge_dst, edge_w, W, b):
    edge_src = np.asarray(edge_src, np.int64)
    edge_dst = np.asarray(edge_dst, np.int64)
    edge_w = np.asarray(edge_w, np.float32)

    core = edge_dst // SHARD
    loc = edge_dst - core * SHARD
    blk = loc >> 7
    dst_local = (loc & 127).astype(np.float32)
    q = edge_src // QROWS
    src_local = (edge_src - q * QROWS).astype(np.int16)

    # per (core, block, q) counts; rank blocks per core by total edges
    cell_cnt = np.zeros((N_CORES, NBLK, NQ), np.int64)
    np.add.at(cell_cnt, (core, blk, q), 1)
    blk_tot = cell_cnt.sum(axis=2)
    perm = np.argsort(-blk_tot, axis=1, kind="stable")   # rank -> orig block
    inv_perm = np.empty_like(perm)
    np.put_along_axis(inv_perm, perm, np.arange(NBLK)[None, :], axis=1)

    ranked_cnt = np.take_along_axis(cell_cnt, perm[:, :, None], axis=1)
    caps = ranked_cnt.max(axis=0)                        # [NBLK, NQ]
    caps16 = np.maximum(16, ((caps + 15) // 16) * 16)

    chunks, qbase, c_rank, rankbase, icols, ibase = _plan(caps16)
    tot_chunks = int(c_rank.sum())
    tot_icols = int(ibase[-1])

    # slot assignment within each (core, rank, q) cell
    rank_e = inv_perm[core, blk]
    cell = ((core * NBLK + rank_e) * NQ + q)
    order = np.argsort(cell, kind="stable")
    cell_s = cell[order]
    counts_s = np.bincount(cell_s, minlength=N_CORES * NBLK * NQ)
    starts = np.zeros(N_CORES * NBLK * NQ, np.int64)
    starts[1:] = np.cumsum(counts_s)[:-1]
    srank = np.arange(len(order)) - starts[cell_s]

    core_s = cell_s // (NBLK * NQ)
    rem = cell_s - core_s * (NBLK * NQ)
    r_s = rem // NQ
    q_s = rem - r_s * NQ

    # gather indices: linear slot stream with pad slots repeating the last
    # real index (DRAM row-buffer hits), wrapped [16, cap/16], replicated x8
    tot_slots = tot_icols * 16
    idx_lin = np.full((N_CORES, tot_slots), -1, np.int32)
    idx_lin[core_s, ibase[rem] * 16 + srank] = src_local[order]
    pos = np.where(idx_lin >= 0, np.arange(tot_slots)[None, :], 0)
    np.maximum.accumulate(pos, axis=1, out=pos)
    idx_lin = np.maximum(np.take_along_axis(idx_lin, pos, axis=1), 0)
    idx_flat = np.ascontiguousarray(
        idx_lin.reshape(N_CORES, tot_icols, 16).transpose(0, 2, 1)
    ).astype(np.int16)
    idx_all = np.tile(idx_flat, (1, 8, 1))

    # per-slot metadata (bf16: dst 0..127 and uniform weights are exact
    # enough for the 2e-2 gate); column-duplicated for the DVE pair trick
    mdst_all = np.zeros((N_CORES, P, tot_chunks), ml_dtypes.bfloat16)
    mw_all = np.zeros((N_CORES, P, tot_chunks), ml_dtypes.bfloat16)
    colpos = rankbase[r_s] + qbase[r_s, q_s] + (srank >> 7)
    part = srank & 127
    mdst_all[core_s, part, colpos] = dst_local[order]
    mw_all[core_s, part, colpos] = edge_w[order]
    mdst_all = np.repeat(mdst_all, 2, axis=2)
    mw_all = np.repeat(mw_all, 2, axis=2)

    bbc = np.broadcast_to(np.asarray(b, np.float32), (P, D_OUT)).copy()
    wmat = np.ascontiguousarray(np.asarray(W, np.float32))
    x_tbl = np.asarray(x, np.float32).astype(ml_dtypes.bfloat16)

    in_maps = []
    for m in range(N_CORES):
        in_maps.append({
            "x_tbl": x_tbl,
            "wmat": wmat,
            "bbc": bbc,
            "idx": np.ascontiguousarray(idx_all[m]),
            "mdst": mdst_all[m],
            "mw": mw_all[m],
        })
    return in_maps, caps16, perm


def kernel(x, edge_src, edge_dst, edge_w, W, b):
    global last_exec_time_ns, last_results
    in_maps, caps16, perm = _prep_inputs(x, edge_src, edge_dst, edge_w, W, b)
    key = tuple(caps16.reshape(-1).tolist())
    if key not in _program_cache:
        _program_cache[key] = _build_program(key)
    nc = _program_cache[key]
    res = run_bass_kernel_spmd(nc, in_maps, list(range(N_CORES)))
    last_exec_time_ns = res.exec_time_ns
    last_results = res
    full = np.empty((N_CORES, SHARD, D_OUT), np.float32)
    for m in range(N_CORES):
        ranked = np.asarray(res.results[m]["out"]).reshape(NBLK, P, D_OUT)
        unperm = np.empty_like(ranked)
        unperm[perm[m]] = ranked
        full[m] = unperm.reshape(OUT_ROWS, D_OUT)[:SHARD]
    return full.reshape(N_NODES, D_OUT)


# revision 15
# speedup vs baseline: 1.0140x; 1.0140x over previous
"""GraphConvolution kernel for Trainium2 (8 NeuronCores, SPMD).

out = segment_sum(edge_w * (x @ W)[edge_src], edge_dst) + b

Strategy (graph/data parallel, dst-sharded):
  - Each core owns a contiguous shard of 12500 destination nodes, split
    into 98 dst blocks of 128. W commutes with segment_sum, so each core
    gathers raw x rows (bf16), accumulates pre[d, :] = sum_e w_e *
    x[src_e, :] per dst block via selection-matrix matmuls in PSUM (f32),
    then applies W per block and adds the bias.
  - The gather uses dma_gather (GPSIMD extended instruction, int16
    indices); x is addressed through 4 quartile tables of 25000 rows.
    Edges are bucketed per (dst block, quartile) cell. The kernel is
    Q7-descriptor-generation bound (~2.4 ns per gathered index once the
    four SWDGE queues are cycled: queue q runs its descgen on Q7 core
    pair (2q, 2q+1) and has its own descriptor ring, so consecutive
    calls on different queues never stall on one ring's drain).
  - x is cast to bf16 on the host (512B rows); selection matrices are
    built in bf16 on DVE with two broadcast tensor_tensor passes per
    (block, quartile) cell; PE accumulates in f32 PSUM.
  - Cell capacities are rounded to 16 indices; pad slots gather row 0
    and are nulled by zero columns (w=0) in the selection matrix. Each
    core orders its dst blocks by descending edge count (the output is
    un-permuted on the host) so the SPMD-uniform per-(rank, quartile)
    cap (max over cores) stays tight.
"""

import ml_dtypes
import numpy as np

import concourse.bass as bass
import concourse.bacc as bacc
import concourse.mybir as mybir
import concourse.tile as tile
from concourse.bass_utils import run_bass_kernel_spmd
from concourse.masks import make_identity

N_NODES = 100000
D_IN = 256
D_OUT = 128
N_CORES = 8
SHARD = N_NODES // N_CORES          # 12500 dst rows per core
P = 128
NBLK = (SHARD + P - 1) // P         # 98 dst blocks per core
OUT_ROWS = NBLK * P                 # 12544 padded output rows per core
NQ = 4
QROWS = (N_NODES + NQ - 1) // NQ    # 25000 rows per quartile table
GBUFS = 5                           # gather-tile buffering

last_exec_time_ns = None
last_results = None
_program_cache = {}


def _plan(caps16):
    """Derive static layout tables from the per-(rank, q) index counts."""
    chunks = (caps16 + P - 1) // P                 # [NBLK, NQ]
    qbase = np.zeros((NBLK, NQ), np.int64)
    qbase[:, 1:] = np.cumsum(chunks, axis=1)[:, :-1]
    c_rank = chunks.sum(axis=1)                    # chunks per ranked block
    rankbase = np.zeros(NBLK, np.int64)
    rankbase[1:] = np.cumsum(c_rank)[:-1]
    icols = caps16 // 16                           # idx columns per call
    ibase = np.zeros(NBLK * NQ + 1, np.int64)
    ibase[1:] = np.cumsum(icols.reshape(-1))
    return chunks, qbase, c_rank, rankbase, icols, ibase


def _build_program(caps_key):
    f32 = mybir.dt.float32
    bf16 = mybir.dt.bfloat16
    i16 = mybir.dt.int16
    caps16 = np.asarray(caps_key, np.int64).reshape(NBLK, NQ)
    chunks, qbase, c_rank, rankbase, icols, ibase = _plan(caps16)
    tot_chunks = int(c_rank.sum())
    tot_icols = int(ibase[-1])
    max_c = int(c_rank.max())
    max_cell = int(chunks.max())

    nc = bacc.Bacc("TRN2", target_bir_lowering=False, debug=False,
                   num_devices=N_CORES, num_swdge_queues=4)
    x_tbl = nc.dram_tensor("x_tbl", [N_NODES, D_IN], bf16,
                           kind="ExternalInput").ap()
    wmat = nc.dram_tensor("wmat", [D_IN, D_OUT], f32, kind="ExternalInput").ap()
    bbc = nc.dram_tensor("bbc", [P, D_OUT], f32, kind="ExternalInput").ap()
    idx = nc.dram_tensor("idx", [P, tot_icols], i16, kind="ExternalInput").ap()
    # mdst/mw are column-duplicated (pairs) so the broadcast APs in the mt
    # build keep an inner step-1 pair dim — required for DVE 2X_1PORT mode
    mdst = nc.dram_tensor("mdst", [P, 2 * tot_chunks], bf16,
                          kind="ExternalInput").ap()
    mw = nc.dram_tensor("mw", [P, 2 * tot_chunks], bf16,
                        kind="ExternalInput").ap()
    out = nc.dram_tensor("out", [OUT_ROWS, D_OUT], f32,
                         kind="ExternalOutput").ap()

    with tile.TileContext(nc) as tc:
        with (
            tc.tile_pool(name="const", bufs=1) as constp,
            tc.tile_pool(name="meta", bufs=1) as metap,
            tc.tile_pool(name="g", bufs=GBUFS) as gp,
            tc.tile_pool(name="eq", bufs=2) as eqp,
            tc.tile_pool(name="m", bufs=2) as mp,
            tc.tile_pool(name="pre", bufs=2, space="PSUM") as prep,
            tc.tile_pool(name="tp", bufs=2, space="PSUM") as tpp,
            tc.tile_pool(name="po", bufs=2, space="PSUM") as pop,
            tc.tile_pool(name="sb", bufs=3) as sbp,
            tc.tile_pool(name="st", bufs=4) as stp,
            tc.tile_pool(name="ob", bufs=3) as obp,
        ):
            w0 = constp.tile([P, D_OUT], f32, tag="w0")
            w1 = constp.tile([P, D_OUT], f32, tag="w1")
            nc.sync.dma_start(out=w0[:], in_=wmat[0:P, :])
            nc.sync.dma_start(out=w1[:], in_=wmat[P:2 * P, :])
            bb = constp.tile([P, D_OUT], f32, tag="bb")
            nc.sync.dma_start(out=bb[:], in_=bbc[:])
            iota_i = constp.tile([P, P], mybir.dt.int32, tag="ioi")
            nc.gpsimd.iota(iota_i[:], pattern=[[1, P]], base=0,
                           channel_multiplier=0)
            iota_b = constp.tile([P, P], bf16, tag="iob")
            nc.vector.tensor_copy(iota_b[:], iota_i[:])
            ident = constp.tile([P, P], f32, tag="id")
            make_identity(nc, ident[:])
            ones1 = constp.tile([1, P], f32, tag="on")
            nc.vector.memset(ones1[:], 1.0)

            idx_t = metap.tile([P, tot_icols], i16, tag="idx")
            mdst_t = metap.tile([P, 2 * tot_chunks], bf16, tag="mdst")
            mw_t = metap.tile([P, 2 * tot_chunks], bf16, tag="mw")
            nc.sync.dma_start(out=idx_t[:], in_=idx[:])
            nc.sync.dma_start(out=mdst_t[:], in_=mdst[:])
            nc.sync.dma_start(out=mw_t[:], in_=mw[:])

            for r in range(NBLK):
                C = int(c_rank[r])
                gt = gp.tile([P, max_c * D_IN], bf16, tag="g")
                for q in range(NQ):
                    cap = int(caps16[r, q])
                    nch = int(chunks[r, q])
                    call = r * NQ + q
                    nc.gpsimd.dma_gather(
                        out_ap=gt[:, qbase[r, q] * D_IN:
                                  (qbase[r, q] + nch) * D_IN]
                        .rearrange("p (c d) -> p c d", d=D_IN),
                        in_ap=x_tbl[q * QROWS:(q + 1) * QROWS, :],
                        idxs_ap=idx_t[:, ibase[call]:ibase[call + 1]],
                        num_idxs=cap, num_idxs_reg=cap,
                        elem_size=D_IN, single_packet=False,
                        queue_num=q,
                    )
                pre = prep.tile([P, D_IN], f32, tag="pre")
                # selection matrices for the whole block, built in two
                # broadcast passes: eq[p, c, d] = (d == mdst[p, col0+c]),
                # mt = eq * mw. The pair-duplicated mdst/mw tables keep every
                # AP's innermost dim at step 1 (DVE 2X_1PORT eligibility).
                col0 = int(rankbase[r])
                eqt = eqp.tile([P, max_c * P], bf16, tag="eq")
                mt = mp.tile([P, max_c * P], bf16, tag="m")
                e4 = eqt[:, :C * P].rearrange("p (c e j) -> p c e j",
                                              e=P // 2, j=2)
                nc.vector.tensor_tensor(
                    out=e4,
                    in0=iota_b[:].rearrange("p (c e j) -> p c e j",
                                            c=1, j=2)
                    .to_broadcast([P, C, P // 2, 2]),
                    in1=mdst_t[:, 2 * col0:2 * (col0 + C)]
                    .rearrange("p (c e j) -> p c e j", e=1, j=2)
                    .to_broadcast([P, C, P // 2, 2]),
                    op=mybir.AluOpType.is_equal,
                )
                m4 = mt[:, :C * P].rearrange("p (c e j) -> p c e j",
                                             e=P // 2, j=2)
                nc.vector.tensor_tensor(
                    out=m4,
                    in0=e4,
                    in1=mw_t[:, 2 * col0:2 * (col0 + C)]
                    .rearrange("p (c e j) -> p c e j", e=1, j=2)
                    .to_broadcast([P, C, P // 2, 2]),
                    op=mybir.AluOpType.mult,
                )
                tot_c = 0
                for q in range(NQ):
                    cap = int(caps16[r, q])
                    nch = int(chunks[r, q])
                    for c in range(nch):
                        K = P if (c < nch - 1 or cap % P == 0) else cap % P
                        gcol = int(qbase[r, q]) + c
                        nc.tensor.matmul(
                            out=pre[:], lhsT=mt[:K, gcol * P:(gcol + 1) * P],
                            rhs=gt[:K, gcol * D_IN:(gcol + 1) * D_IN],
                            start=(tot_c == 0), stop=(tot_c == C - 1),
                        )
                        tot_c += 1
                assert tot_c == C
                sb_pre = sbp.tile([P, D_IN], f32, tag="sb")
                nc.scalar.copy(sb_pre[:], pre[:])
                po = pop.tile([P, D_OUT], f32, tag="po")
                nc.tensor.matmul(out=po[:], lhsT=ones1[:],
                                 rhs=bb[0:1, :], start=True, stop=False)
                for h in range(2):
                    pt = tpp.tile([P, P], f32, tag="pt")
                    nc.tensor.transpose(pt[:], sb_pre[:, h * P:(h + 1) * P],
                                        ident[:])
                    st = stp.tile([P, P], f32, tag="st")
                    nc.scalar.copy(st[:], pt[:])
                    nc.tensor.matmul(out=po[:], lhsT=st[:],
                                     rhs=(w0[:] if h == 0 else w1[:]),
                                     start=False, stop=(h == 1))
                ob = obp.tile([P, D_OUT], f32, tag="ob")
                nc.scalar.copy(ob[:], po[:])
                nc.sync.dma_start(out=out[r * P:(r + 1) * P, :], in_=ob[:])

    nc.compile()
    return nc


def _prep_inputs(x, edge_src, edge_dst, edge_w, W, b):
    edge_src = np.asarray(edge_src, np.int64)
    edge_dst = np.asarray(edge_dst, np.int64)
    edge_w = np.asarray(edge_w, np.float32)

    core = edge_dst // SHARD
    loc = edge_dst - core * SHARD
    blk = loc >> 7
    dst_local = (loc & 127).astype(np.float32)
    q = edge_src // QROWS
    src_local = (edge_src - q * QROWS).astype(np.int16)

    # per (core, block, q) counts; rank blocks per core by total edges
    cell_cnt = np.zeros((N_CORES, NBLK, NQ), np.int64)
    np.add.at(cell_cnt, (core, blk, q), 1)
    blk_tot = cell_cnt.sum(axis=2)
    perm = np.argsort(-blk_tot, axis=1, kind="stable")   # rank -> orig block
    inv_perm = np.empty_like(perm)
    np.put_along_axis(inv_perm, perm, np.arange(NBLK)[None, :], axis=1)

    ranked_cnt = np.take_along_axis(cell_cnt, perm[:, :, None], axis=1)
    caps = ranked_cnt.max(axis=0)                        # [NBLK, NQ]
    caps16 = np.maximum(16, ((caps + 15) // 16) * 16)

    chunks, qbase, c_rank, rankbase, icols, ibase = _plan(caps16)
    tot_chunks = int(c_rank.sum())
    tot_icols = int(ibase[-1])

    # slot assignment within each (core, rank, q) cell
    rank_e = inv_perm[core, blk]
    cell = ((core * NBLK + rank_e) * NQ + q)
    order = np.argsort(cell, kind="stable")
    cell_s = cell[order]
    counts_s = np.bincount(cell_s, minlength=N_CORES * NBLK * NQ)
    starts = np.zeros(N_CORES * NBLK * NQ, np.int64)
    starts[1:] = np.cumsum(counts_s)[:-1]
    srank = np.arange(len(order)) - starts[cell_s]

    core_s = cell_s // (NBLK * NQ)
    rem = cell_s - core_s * (NBLK * NQ)
    r_s = rem // NQ
    q_s = rem - r_s * NQ

    # gather indices: per call, wrapped [16, cap/16] then replicated x8
    idx_flat = np.zeros((N_CORES, 16, tot_icols), np.int16)
    jpos = srank
    idx_flat[core_s, jpos % 16, ibase[rem] + jpos // 16] = src_local[order]
    idx_all = np.tile(idx_flat, (1, 8, 1))

    # per-slot metadata (bf16: dst 0..127 and uniform weights are exact
    # enough for the 2e-2 gate); column-duplicated for the DVE pair trick
    mdst_all = np.zeros((N_CORES, P, tot_chunks), ml_dtypes.bfloat16)
    mw_all = np.zeros((N_CORES, P, tot_chunks), ml_dtypes.bfloat16)
    colpos = rankbase[r_s] + qbase[r_s, q_s] + (srank >> 7)
    part = srank & 127
    mdst_all[core_s, part, colpos] = dst_local[order]
    mw_all[core_s, part, colpos] = edge_w[order]
    mdst_all = np.repeat(mdst_all, 2, axis=2)
    mw_all = np.repeat(mw_all, 2, axis=2)

    bbc = np.broadcast_to(np.asarray(b, np.float32), (P, D_OUT)).copy()
    wmat = np.ascontiguousarray(np.asarray(W, np.float32))
    x_tbl = np.asarray(x, np.float32).astype(ml_dtypes.bfloat16)

    in_maps = []
    for m in range(N_CORES):
        in_maps.append({
            "x_tbl": x_tbl,
            "wmat": wmat,
            "bbc": bbc,
            "idx": np.ascontiguousarray(idx_all[m]),
            "mdst": mdst_all[m],
            "mw": mw_all[m],
        })
    return in_maps, caps16, perm


def kernel(x, edge_src, edge_dst, edge_w, W, b):
    global last_exec_time_ns, last_results
    in_maps, caps16, perm = _prep_inputs(x, edge_src, edge_dst, edge_w, W, b)
    key = tuple(caps16.reshape(-1).tolist())
    if key not in _program_cache:
        _program_cache[key] = _build_program(key)
    nc = _program_cache[key]
    res = run_bass_kernel_spmd(nc, in_maps, list(range(N_CORES)))
    last_exec_time_ns = res.exec_time_ns
    last_results = res
    full = np.empty((N_CORES, SHARD, D_OUT), np.float32)
    for m in range(N_CORES):
        ranked = np.asarray(res.results[m]["out"]).reshape(NBLK, P, D_OUT)
        unperm = np.empty_like(ranked)
        unperm[perm[m]] = ranked
        full[m] = unperm.reshape(OUT_ROWS, D_OUT)[:SHARD]
    return full.reshape(N_NODES, D_OUT)


# revision 25
# speedup vs baseline: 1.0229x; 1.0088x over previous
"""GraphConvolution kernel for Trainium2 (8 NeuronCores, SPMD).

out = segment_sum(edge_w * (x @ W)[edge_src], edge_dst) + b

Strategy (graph/data parallel, dst-sharded):
  - Each core owns a contiguous shard of 12500 destination nodes, split
    into 98 dst blocks of 128. W commutes with segment_sum, so each core
    gathers raw x rows (bf16), accumulates pre[d, :] = sum_e w_e *
    x[src_e, :] per dst block via selection-matrix matmuls in PSUM (f32),
    then applies W per block and adds the bias.
  - The gather uses dma_gather (GPSIMD extended instruction, int16
    indices); x is addressed through 4 quartile tables of 25000 rows.
    Edges are bucketed per (dst block, quartile) cell. The kernel is
    Q7-descriptor-generation bound (~2.4 ns per gathered index once the
    four SWDGE queues are cycled: queue q runs its descgen on Q7 core
    pair (2q, 2q+1) and has its own descriptor ring, so consecutive
    calls on different queues never stall on one ring's drain).
  - x is cast to bf16 on the host (512B rows); selection matrices are
    built in bf16 on DVE with two broadcast tensor_tensor passes per
    (block, quartile) cell; PE accumulates in f32 PSUM.
  - Cell capacities are rounded to 16 indices; pad slots gather row 0
    and are nulled by zero columns (w=0) in the selection matrix. Each
    core orders its dst blocks by descending edge count (the output is
    un-permuted on the host) so the SPMD-uniform per-(rank, quartile)
    cap (max over cores) stays tight.
"""

import ml_dtypes
import numpy as np

import concourse.bass as bass
import concourse.bacc as bacc
import concourse.mybir as mybir
import concourse.tile as tile
from concourse.bass_utils import run_bass_kernel_spmd
from concourse.masks import make_identity

N_NODES = 100000
D_IN = 256
D_OUT = 128
N_CORES = 8
SHARD = N_NODES // N_CORES          # 12500 dst rows per core
P = 128
NBLK = (SHARD + P - 1) // P         # 98 dst blocks per core
OUT_ROWS = NBLK * P                 # 12544 padded output rows per core
NQ = 4
QROWS = (N_NODES + NQ - 1) // NQ    # 25000 rows per quartile table
GBUFS = 5                           # gather-tile buffering

last_exec_time_ns = None
last_results = None
_program_cache = {}


def _plan(caps16):
    """Derive static layout tables from the per-(rank, q) index counts."""
    chunks = (caps16 + P - 1) // P                 # [NBLK, NQ]
    qbase = np.zeros((NBLK, NQ), np.int64)
    qbase[:, 1:] = np.cumsum(chunks, axis=1)[:, :-1]
    c_rank = chunks.sum(axis=1)                    # chunks per ranked block
    rankbase = np.zeros(NBLK, np.int64)
    rankbase[1:] = np.cumsum(c_rank)[:-1]
    icols = caps16 // 16                           # idx columns per call
    ibase = np.zeros(NBLK * NQ + 1, np.int64)
    ibase[1:] = np.cumsum(icols.reshape(-1))
    return chunks, qbase, c_rank, rankbase, icols, ibase


def _build_program(caps_key):
    f32 = mybir.dt.float32
    bf16 = mybir.dt.bfloat16
    i16 = mybir.dt.int16
    caps16 = np.asarray(caps_key, np.int64).reshape(NBLK, NQ)
    chunks, qbase, c_rank, rankbase, icols, ibase = _plan(caps16)
    tot_chunks = int(c_rank.sum())
    tot_icols = int(ibase[-1])
    max_c = int(c_rank.max())
    max_cell = int(chunks.max())

    nc = bacc.Bacc("TRN2", target_bir_lowering=False, debug=False,
                   num_devices=N_CORES, num_swdge_queues=4)
    x_tbl = nc.dram_tensor("x_tbl", [N_NODES, D_IN], bf16,
                           kind="ExternalInput").ap()
    wmat = nc.dram_tensor("wmat", [D_IN, D_OUT], bf16,
                          kind="ExternalInput").ap()
    bbc = nc.dram_tensor("bbc", [P, D_OUT], bf16, kind="ExternalInput").ap()
    idx = nc.dram_tensor("idx", [P, tot_icols], i16, kind="ExternalInput").ap()
    # mdst/mw are column-duplicated (pairs) so the broadcast APs in the mt
    # build keep an inner step-1 pair dim — required for DVE 2X_1PORT mode
    mdst = nc.dram_tensor("mdst", [P, 2 * tot_chunks], bf16,
                          kind="ExternalInput").ap()
    mw = nc.dram_tensor("mw", [P, 2 * tot_chunks], bf16,
                        kind="ExternalInput").ap()
    out = nc.dram_tensor("out", [OUT_ROWS, D_OUT], f32,
                         kind="ExternalOutput").ap()

    with tile.TileContext(nc) as tc:
        with (
            tc.tile_pool(name="const", bufs=1) as constp,
            tc.tile_pool(name="meta", bufs=1) as metap,
            tc.tile_pool(name="g", bufs=GBUFS) as gp,
            tc.tile_pool(name="eq", bufs=2) as eqp,
            tc.tile_pool(name="m", bufs=2) as mp,
            tc.tile_pool(name="pre", bufs=2, space="PSUM") as prep,
            tc.tile_pool(name="tp", bufs=2, space="PSUM") as tpp,
            tc.tile_pool(name="po", bufs=2, space="PSUM") as pop,
            tc.tile_pool(name="sb", bufs=3) as sbp,
            tc.tile_pool(name="st", bufs=4) as stp,
            tc.tile_pool(name="ob", bufs=3) as obp,
        ):
            w0 = constp.tile([P, D_OUT], bf16, tag="w0")
            w1 = constp.tile([P, D_OUT], bf16, tag="w1")
            nc.sync.dma_start(out=w0[:], in_=wmat[0:P, :])
            nc.sync.dma_start(out=w1[:], in_=wmat[P:2 * P, :])
            bb = constp.tile([P, D_OUT], bf16, tag="bb")
            nc.sync.dma_start(out=bb[:], in_=bbc[:])
            iota_i = constp.tile([P, P], mybir.dt.int32, tag="ioi")
            nc.gpsimd.iota(iota_i[:], pattern=[[1, P]], base=0,
                           channel_multiplier=0)
            iota_b = constp.tile([P, P], bf16, tag="iob")
            nc.vector.tensor_copy(iota_b[:], iota_i[:])
            ident = constp.tile([P, P], bf16, tag="id")
            make_identity(nc, ident[:])
            ones1 = constp.tile([1, P], bf16, tag="on")
            nc.vector.memset(ones1[:], 1.0)

            idx_t = metap.tile([P, tot_icols], i16, tag="idx")
            mdst_t = metap.tile([P, 2 * tot_chunks], bf16, tag="mdst")
            mw_t = metap.tile([P, 2 * tot_chunks], bf16, tag="mw")
            nc.sync.dma_start(out=idx_t[:], in_=idx[:])
            nc.sync.dma_start(out=mdst_t[:], in_=mdst[:])
            nc.sync.dma_start(out=mw_t[:], in_=mw[:])

            for r in range(NBLK):
                C = int(c_rank[r])
                gt = gp.tile([P, max_c * D_IN], bf16, tag="g")
                for q in range(NQ):
                    cap = int(caps16[r, q])
                    nch = int(chunks[r, q])
                    call = r * NQ + q
                    nc.gpsimd.dma_gather(
                        out_ap=gt[:, qbase[r, q] * D_IN:
                                  (qbase[r, q] + nch) * D_IN]
                        .rearrange("p (c d) -> p c d", d=D_IN),
                        in_ap=x_tbl[q * QROWS:(q + 1) * QROWS, :],
                        idxs_ap=idx_t[:, ibase[call]:ibase[call + 1]],
                        num_idxs=cap, num_idxs_reg=cap,
                        elem_size=D_IN, single_packet=False,
                        queue_num=q,
                    )
                pre = prep.tile([P, D_IN], f32, tag="pre")
                # selection matrices for the whole block, built in two
                # broadcast passes: eq[p, c, d] = (d == mdst[p, col0+c]),
                # mt = eq * mw. The pair-duplicated mdst/mw tables keep every
                # AP's innermost dim at step 1 (DVE 2X_1PORT eligibility).
                col0 = int(rankbase[r])
                eqt = eqp.tile([P, max_c * P], bf16, tag="eq")
                mt = mp.tile([P, max_c * P], bf16, tag="m")
                e4 = eqt[:, :C * P].rearrange("p (c e j) -> p c e j",
                                              e=P // 2, j=2)
                nc.vector.tensor_tensor(
                    out=e4,
                    in0=iota_b[:].rearrange("p (c e j) -> p c e j",
                                            c=1, j=2)
                    .to_broadcast([P, C, P // 2, 2]),
                    in1=mdst_t[:, 2 * col0:2 * (col0 + C)]
                    .rearrange("p (c e j) -> p c e j", e=1, j=2)
                    .to_broadcast([P, C, P // 2, 2]),
                    op=mybir.AluOpType.is_equal,
                )
                m4 = mt[:, :C * P].rearrange("p (c e j) -> p c e j",
                                             e=P // 2, j=2)
                nc.vector.tensor_tensor(
                    out=m4,
                    in0=e4,
                    in1=mw_t[:, 2 * col0:2 * (col0 + C)]
                    .rearrange("p (c e j) -> p c e j", e=1, j=2)
                    .to_broadcast([P, C, P // 2, 2]),
                    op=mybir.AluOpType.mult,
                )
                tot_c = 0
                for q in range(NQ):
                    cap = int(caps16[r, q])
                    nch = int(chunks[r, q])
                    for c in range(nch):
                        K = P if (c < nch - 1 or cap % P == 0) else cap % P
                        gcol = int(qbase[r, q]) + c
                        nc.tensor.matmul(
                            out=pre[:], lhsT=mt[:K, gcol * P:(gcol + 1) * P],
                            rhs=gt[:K, gcol * D_IN:(gcol + 1) * D_IN],
                            start=(tot_c == 0), stop=(tot_c == C - 1),
                        )
                        tot_c += 1
                assert tot_c == C
                sb_pre = sbp.tile([P, D_IN], bf16, tag="sb")
                nc.scalar.copy(sb_pre[:], pre[:])
                po = pop.tile([P, D_OUT], f32, tag="po")
                nc.tensor.matmul(out=po[:], lhsT=ones1[:],
                                 rhs=bb[0:1, :], start=True, stop=False)
                for h in range(2):
                    pt = tpp.tile([P, P], bf16, tag="pt")
                    nc.tensor.transpose(pt[:], sb_pre[:, h * P:(h + 1) * P],
                                        ident[:])
                    st = stp.tile([P, P], bf16, tag="st")
                    nc.scalar.copy(st[:], pt[:])
                    nc.tensor.matmul(out=po[:], lhsT=st[:],
                                     rhs=(w0[:] if h == 0 else w1[:]),
                                     start=False, stop=(h == 1))
                ob = obp.tile([P, D_OUT], f32, tag="ob")
                nc.scalar.copy(ob[:], po[:])
                nc.sync.dma_start(out=out[r * P:(r + 1) * P, :], in_=ob[:])

    nc.compile()
    return nc


def _prep_inputs(x, edge_src, edge_dst, edge_w, W, b):
    edge_src = np.asarray(edge_src, np.int64)
    edge_dst = np.asarray(edge_dst, np.int64)
    edge_w = np.asarray(edge_w, np.float32)

    core = edge_dst // SHARD
    loc = edge_dst - core * SHARD
    blk = loc >> 7
    dst_local = (loc & 127).astype(np.float32)
    q = edge_src // QROWS
    src_local = (edge_src - q * QROWS).astype(np.int16)

    # per (core, block, q) counts; rank blocks per core by total edges
    cell_cnt = np.zeros((N_CORES, NBLK, NQ), np.int64)
    np.add.at(cell_cnt, (core, blk, q), 1)
    blk_tot = cell_cnt.sum(axis=2)
    perm = np.argsort(-blk_tot, axis=1, kind="stable")   # rank -> orig block
    inv_perm = np.empty_like(perm)
    np.put_along_axis(inv_perm, perm, np.arange(NBLK)[None, :], axis=1)

    ranked_cnt = np.take_along_axis(cell_cnt, perm[:, :, None], axis=1)
    caps = ranked_cnt.max(axis=0)                        # [NBLK, NQ]
    caps16 = np.maximum(16, ((caps + 15) // 16) * 16)

    chunks, qbase, c_rank, rankbase, icols, ibase = _plan(caps16)
    tot_chunks = int(c_rank.sum())
    tot_icols = int(ibase[-1])

    # slot assignment within each (core, rank, q) cell
    rank_e = inv_perm[core, blk]
    cell = ((core * NBLK + rank_e) * NQ + q)
    order = np.argsort(cell, kind="stable")
    cell_s = cell[order]
    counts_s = np.bincount(cell_s, minlength=N_CORES * NBLK * NQ)
    starts = np.zeros(N_CORES * NBLK * NQ, np.int64)
    starts[1:] = np.cumsum(counts_s)[:-1]
    srank = np.arange(len(order)) - starts[cell_s]

    core_s = cell_s // (NBLK * NQ)
    rem = cell_s - core_s * (NBLK * NQ)
    r_s = rem // NQ
    q_s = rem - r_s * NQ

    # gather indices: per call, wrapped [16, cap/16] then replicated x8
    idx_flat = np.zeros((N_CORES, 16, tot_icols), np.int16)
    jpos = srank
    idx_flat[core_s, jpos % 16, ibase[rem] + jpos // 16] = src_local[order]
    idx_flat = np.tile(idx_flat, (1, 8, 1))

    # per-slot metadata (bf16: dst 0..127 and uniform weights are exact
    # enough for the 2e-2 gate); column-duplicated for the DVE pair trick
    mdst_all = np.zeros((N_CORES, P, tot_chunks), ml_dtypes.bfloat16)
    mw_all = np.zeros((N_CORES, P, tot_chunks), ml_dtypes.bfloat16)
    colpos = rankbase[r_s] + qbase[r_s, q_s] + (srank >> 7)
    part = srank & 127
    mdst_all[core_s, part, colpos] = dst_local[order]
    mw_all[core_s, part, colpos] = edge_w[order]
    mdst_all = np.repeat(mdst_all, 2, axis=2)
    mw_all = np.repeat(mw_all, 2, axis=2)

    bbc = np.broadcast_to(
        np.asarray(b, np.float32).astype(ml_dtypes.bfloat16), (P, D_OUT)
    ).copy()
    wmat = np.ascontiguousarray(
        np.asarray(W, np.float32).astype(ml_dtypes.bfloat16))
    x_tbl = np.asarray(x, np.float32).astype(ml_dtypes.bfloat16)

    in_maps = []
    for m in range(N_CORES):
        in_maps.append({
            "x_tbl": x_tbl,
            "wmat": wmat,
            "bbc": bbc,
            "idx": np.ascontiguousarray(idx_flat[m]),
            "mdst": mdst_all[m],
            "mw": mw_all[m],
        })
    return in_maps, caps16, perm


def kernel(x, edge_src, edge_dst, edge_w, W, b):
    global last_exec_time_ns, last_results
    in_maps, caps16, perm = _prep_inputs(x, edge_src, edge_dst, edge_w, W, b)
    key = tuple(caps16.reshape(-1).tolist())
    if key not in _program_cache:
        _program_cache[key] = _build_program(key)
    nc = _program_cache[key]
    res = run_bass_kernel_spmd(nc, in_maps, list(range(N_CORES)))
    last_exec_time_ns = res.exec_time_ns
    last_results = res
    full = np.empty((N_CORES, SHARD, D_OUT), np.float32)
    for m in range(N_CORES):
        ranked = np.asarray(res.results[m]["out"]).reshape(NBLK, P, D_OUT)
        unperm = np.empty_like(ranked)
        unperm[perm[m]] = ranked
        full[m] = unperm.reshape(OUT_ROWS, D_OUT)[:SHARD]
    return full.reshape(N_NODES, D_OUT)
